# revision 2
# baseline (speedup 1.0000x reference)
"""Trainium2 Bass kernel for nn_MockAttentionHead.

Math note: the reference's final steps are
    scores = softmax(sims*temp); scores *= scale; scores /= (rowsum(scores)+eps)
Since softmax rows sum to 1, the scale multiplication cancels in the final
renormalization up to ~eps/scale ~ 1e-10 relative, so the output equals
exp(temp*sims) row-normalized.  The entire score_dists / input_dists / scale
computation has no effect on the output beyond 1e-7 (verified numerically vs
the jax reference: max rel err 1.4e-6, fp32 noise level).

The [B,D,D] metric tensors also reduce analytically: for m = qq^T/D + I,
  fro = sqrt((s/D+1)^2 + D-1),  q^T m q = s*(s/D+1),   with s = ||q||^2,
so q_norm = sqrt(s*(s/D+1)/(fro+eps)) -- all per-row scalar math.

Sharding: data-parallel over query rows; 512 rows per core, key side
replicated.  No collectives.
"""

import sys
import numpy as np

sys.path.insert(0, "/opt/trn_rl_repo")

import concourse.bass as bass
import concourse.mybir as mybir
import concourse.tile as tile
from concourse.masks import make_identity

B = 4096
D = 128
NCORES = 8
R = B // NCORES          # 512 query rows per core
IT = R // 128            # 4 i-tiles per core
JTS = B // 128           # 32 j-tiles (128 wide)
JC = B // 512            # 8 j-chunks (512 wide)
TEMP = float(np.sqrt(float(D)))
EPS = 1e-8

F32 = mybir.dt.float32
BF16 = mybir.dt.bfloat16
MUL = mybir.AluOpType.mult
ADD = mybir.AluOpType.add
AX_X = mybir.AxisListType.X
SQRT = mybir.ActivationFunctionType.Sqrt
EXPF = mybir.ActivationFunctionType.Exp
SQUARE = mybir.ActivationFunctionType.Square


def _norm_chain(nc, pool, s, n, label):
    """Per-partition metric-norm chain on a packed [128, n] tile `s` of row
    norms ||x||^2.  Returns (inv_norm u, a = s*u^2) tiles [128, n]."""
    t = pool.tile([128, n], F32, name=f"t_{label}", tag=f"t_{label}")
    nc.vector.tensor_scalar(t, s, 1.0 / D, 1.0, MUL, ADD)          # t = s/D + 1
    t2 = pool.tile([128, n], F32, name=f"t2_{label}", tag=f"t2_{label}")
    nc.vector.tensor_mul(t2, t, t)                                  # t^2
    cD1 = pool.tile([128, 1], F32, name=f"cD1_{label}", tag=f"cD1_{label}")
    nc.vector.memset(cD1, float(D - 1))
    fro = pool.tile([128, n], F32, name=f"fro_{label}", tag=f"fro_{label}")
    nc.scalar.activation(fro, t2, SQRT, bias=cD1[:, 0:1])           # sqrt(t^2+D-1)
    nc.vector.tensor_scalar_add(fro, fro, EPS)
    rec = pool.tile([128, n], F32, name=f"rec_{label}", tag=f"rec_{label}")
    nc.vector.reciprocal(rec, fro)
    num = pool.tile([128, n], F32, name=f"num_{label}", tag=f"num_{label}")
    nc.vector.tensor_mul(num, s, t)                                 # s*t
    nc.vector.tensor_mul(num, num, rec)                             # s*t/(fro+eps)
    qn = pool.tile([128, n], F32, name=f"qn_{label}", tag=f"qn_{label}")
    nc.scalar.activation(qn, num, SQRT)                             # the metric norm
    nc.vector.tensor_scalar_add(qn, qn, EPS)
    u = pool.tile([128, n], F32, name=f"u_{label}", tag=f"u_{label}")
    nc.vector.reciprocal(u, qn)                                     # 1/(norm+eps)
    a = pool.tile([128, n], F32, name=f"a_{label}", tag=f"a_{label}")
    nc.vector.tensor_mul(a, u, u)
    nc.vector.tensor_mul(a, s, a)                                   # s*u^2 = ||xn||^2
    return u, a


def _trace(nc, with_bias):
    qT = nc.dram_tensor("qT", [D, R], F32, kind="ExternalInput").ap()
    kT = nc.dram_tensor("kT", [D, B], F32, kind="ExternalInput").ap()
    wqT = nc.dram_tensor("wqT", [D, D], F32, kind="ExternalInput").ap()
    wkT = nc.dram_tensor("wkT", [D, D], F32, kind="ExternalInput").ap()
    if with_bias:
        bq_row = nc.dram_tensor("bq_row", [1, D], F32, kind="ExternalInput").ap()
        bk_row = nc.dram_tensor("bk_row", [1, D], F32, kind="ExternalInput").ap()
    out = nc.dram_tensor("out", [R, B], F32, kind="ExternalOutput").ap()

    with tile.TileContext(nc) as tc:
        from contextlib import ExitStack

        ctx = ExitStack()
        with ctx:
            consts = ctx.enter_context(tc.tile_pool(name="consts", bufs=1))
            work = ctx.enter_context(tc.tile_pool(name="work", bufs=1))
            scratch = ctx.enter_context(tc.tile_pool(name="scratch", bufs=4))
            ps_proj = ctx.enter_context(
                tc.tile_pool(name="ps_proj", bufs=2, space="PSUM"))
            ps_tr = ctx.enter_context(
                tc.tile_pool(name="ps_tr", bufs=2, space="PSUM"))
            ps_main = ctx.enter_context(
                tc.tile_pool(name="ps_main", bufs=4, space="PSUM"))

            ident = consts.tile([128, 128], F32, name="ident")
            make_identity(nc, ident)
            ones2 = consts.tile([2, 128], BF16, name="ones2")
            nc.vector.memset(ones2, 1.0)

            wq_s = consts.tile([D, D], F32, name="wq_s")
            nc.sync.dma_start(out=wq_s, in_=wqT)
            wk_s = consts.tile([D, D], F32, name="wk_s")
            nc.sync.dma_start(out=wk_s, in_=wkT)
            qT_s = consts.tile([D, R], F32, name="qT_s")
            nc.sync.dma_start(out=qT_s, in_=qT)
            kT_s = consts.tile([D, B], F32, name="kT_s")
            nc.sync.dma_start(out=kT_s, in_=kT)
            if with_bias:
                ones1 = consts.tile([1, 128], F32, name="ones1")
                nc.vector.memset(ones1, 1.0)
                bq_s = consts.tile([1, D], F32, name="bq_s")
                nc.sync.dma_start(out=bq_s, in_=bq_row)
                bk_s = consts.tile([1, D], F32, name="bk_s")
                nc.sync.dma_start(out=bk_s, in_=bk_row)

            # ---- projections (row layout), row norms --------------------
            # k side first: its chain feeds the bhl rows needed by the main
            # loop's second accumulation matmul.
            k_rows = []
            s_k = work.tile([128, JTS], F32, name="s_k")
            for jt in range(JTS):
                ps = ps_proj.tile([128, 128], F32, name=f"pk{jt}", tag="ps_proj")
                nc.tensor.matmul(ps, lhsT=kT_s[:, jt * 128:(jt + 1) * 128],
                                 rhs=wk_s, start=True, stop=not with_bias)
                if with_bias:
                    nc.tensor.matmul(ps, lhsT=ones1, rhs=bk_s,
                                     start=False, stop=True)
                kr = work.tile([128, 128], F32, name=f"kr{jt}", tag=f"kr{jt}")
                nc.vector.tensor_copy(kr, ps)
                k_rows.append(kr)
                sq = scratch.tile([128, 128], F32, name=f"sqk{jt}", tag="sq_scr")
                nc.scalar.activation(sq, ps, SQUARE,
                                     accum_out=s_k[:, jt:jt + 1])

            v_k, b_k = _norm_chain(nc, work, s_k, JTS, "k")
            vm2 = work.tile([128, JTS], F32, name="vm2")
            nc.vector.tensor_scalar_mul(vm2, v_k, -2.0)

            # b_j split hi/lo bf16, transposed to a [2, B] free-layout tile
            bhi16 = work.tile([128, JTS], BF16, name="bhi16")
            nc.vector.tensor_copy(bhi16, b_k)
            bhi32 = work.tile([128, JTS], F32, name="bhi32")
            nc.vector.tensor_copy(bhi32, bhi16)
            blo32 = work.tile([128, JTS], F32, name="blo32")
            nc.vector.tensor_sub(blo32, b_k, bhi32)
            bhl = work.tile([2, B], BF16, name="bhl")
            for src, row, nm in ((bhi32, 0, "hi"), (blo32, 1, "lo")):
                pst = ps_tr.tile([JTS, 128], F32, name=f"psb_{nm}", tag="ps_tr")
                nc.tensor.transpose(pst, src, ident)
                sb16 = work.tile([JTS, 128], BF16, name=f"sb16_{nm}")
                nc.vector.tensor_copy(sb16, pst)
                nc.sync.dma_start(out=bhl[row:row + 1, :], in_=sb16)

            # scaled kn2 = -2 * v_j * k_rows, transposed into ksT2 [D, B]
            ksT2 = work.tile([D, B], F32, name="ksT2")
            for jt in range(JTS):
                kn2 = scratch.tile([128, 128], F32, name=f"kn2_{jt}", tag="kn2")
                nc.vector.tensor_scalar_mul(kn2, k_rows[jt], vm2[:, jt:jt + 1])
                pst = ps_tr.tile([128, 128], F32, name=f"pskt{jt}", tag="ps_tr")
                nc.tensor.transpose(pst, kn2, ident)
                nc.vector.tensor_copy(ksT2[:, jt * 128:(jt + 1) * 128], pst)

            # q side
            q_rows = []
            s_q = work.tile([128, IT], F32, name="s_q")
            for it in range(IT):
                ps = ps_proj.tile([128, 128], F32, name=f"pq{it}", tag="ps_proj")
                nc.tensor.matmul(ps, lhsT=qT_s[:, it * 128:(it + 1) * 128],
                                 rhs=wq_s, start=True, stop=not with_bias)
                if with_bias:
                    nc.tensor.matmul(ps, lhsT=ones1, rhs=bq_s,
                                     start=False, stop=True)
                qr = work.tile([128, 128], F32, name=f"qr{it}", tag=f"qr{it}")
                nc.vector.tensor_copy(qr, ps)
                q_rows.append(qr)
                sq = scratch.tile([128, 128], F32, name=f"sqq{it}", tag="sq_scr")
                nc.scalar.activation(sq, ps, SQUARE,
                                     accum_out=s_q[:, it:it + 1])

            u_q, a_q = _norm_chain(nc, work, s_q, IT, "q")

            qsT = work.tile([D, R], F32, name="qsT")
            for it in range(IT):
                qn1 = scratch.tile([128, 128], F32, name=f"qn1_{it}", tag="kn2")
                nc.vector.tensor_scalar_mul(qn1, q_rows[it], u_q[:, it:it + 1])
                pst = ps_tr.tile([128, 128], F32, name=f"psqt{it}", tag="ps_tr")
                nc.tensor.transpose(pst, qn1, ident)
                nc.vector.tensor_copy(qsT[:, it * 128:(it + 1) * 128], pst)

            # ---- main loop: d2 -> sqrt -> 1/(1+d) ----------------------
            r_tiles = [[None] * JC for _ in range(IT)]
            for it in range(IT):
                for jc in range(JC):
                    ps = ps_main.tile([128, 512], F32,
                                      name=f"pm{it}_{jc}", tag="ps_main")
                    nc.tensor.matmul(
                        ps, lhsT=qsT[:, it * 128:(it + 1) * 128],
                        rhs=ksT2[:, jc * 512:(jc + 1) * 512],
                        start=True, stop=False)
                    nc.tensor.matmul(
                        ps, lhsT=ones2, rhs=bhl[:, jc * 512:(jc + 1) * 512],
                        start=False, stop=True)
                    rt = work.tile([128, 512], F32,
                                   name=f"r{it}_{jc}", tag=f"r{it}_{jc}")
                    # d = sqrt(-2 qn.kn + b_j + a_i)
                    nc.scalar.activation(rt, ps, SQRT, bias=a_q[:, it:it + 1])
                    nc.gpsimd.tensor_scalar_add(rt, rt, 1.0)      # 1 + d
                    nc.vector.reciprocal(rt, rt)                  # sims = 1/(1+d)
                    r_tiles[it][jc] = rt

            # ---- exp + row sums ----------------------------------------
            rowsums = []
            for it in range(IT):
                rs = work.tile([128, JC], F32, name=f"rs{it}")
                rowsums.append(rs)
                for jc in range(JC):
                    rt = r_tiles[it][jc]
                    nc.scalar.activation(rt, rt, EXPF, scale=TEMP,
                                         accum_out=rs[:, jc:jc + 1])

            # ---- normalize rows and store ------------------------------
            for it in range(IT):
                tot = work.tile([128, 1], F32, name=f"tot{it}")
                nc.vector.reduce_sum(tot, rowsums[it], axis=AX_X, op=ADD)
                inv = work.tile([128, 1], F32, name=f"inv{it}")
                nc.vector.reciprocal(inv, tot)
                for jc in range(JC):
                    rt = r_tiles[it][jc]
                    nc.vector.tensor_scalar_mul(rt, rt, inv[:, 0:1])
                    nc.sync.dma_start(
                        out=out[it * 128:(it + 1) * 128,
                                jc * 512:(jc + 1) * 512],
                        in_=rt)
    return nc


_NC_CACHE = {}


def _get_nc(with_bias):
    if with_bias not in _NC_CACHE:
        from concourse import bacc
        nc = bacc.Bacc("TRN2", target_bir_lowering=False, debug=False)
        _trace(nc, with_bias)
        nc.compile()
        _NC_CACHE[with_bias] = nc
    return _NC_CACHE[with_bias]


def _in_maps(query_points, key_points, Wq, bq, Wk, bk, with_bias):
    qT = np.ascontiguousarray(query_points.T.astype(np.float32, copy=False))
    kT = np.ascontiguousarray(key_points.T.astype(np.float32, copy=False))
    wqT = np.ascontiguousarray(Wq.T.astype(np.float32, copy=False))
    wkT = np.ascontiguousarray(Wk.T.astype(np.float32, copy=False))
    maps = []
    for c in range(NCORES):
        m = {
            "qT": np.ascontiguousarray(qT[:, c * R:(c + 1) * R]),
            "kT": kT,
            "wqT": wqT,
            "wkT": wkT,
        }
        if with_bias:
            m["bq_row"] = np.ascontiguousarray(
                bq.astype(np.float32, copy=False).reshape(1, D))
            m["bk_row"] = np.ascontiguousarray(
                bk.astype(np.float32, copy=False).reshape(1, D))
        maps.append(m)
    return maps


LAST_EXEC_NS = None


def run(query_points, key_points, Wq, bq, Wk, bk, trace=False):
    global LAST_EXEC_NS
    with_bias = bool(np.any(bq) or np.any(bk))
    nc = _get_nc(with_bias)
    maps = _in_maps(query_points, key_points, Wq, bq, Wk, bk, with_bias)
    from concourse import bass_utils
    res = bass_utils.run_bass_kernel_spmd(
        nc, maps, core_ids=list(range(NCORES)), trace=trace)
    LAST_EXEC_NS = res.exec_time_ns
    out = np.concatenate([res.results[c]["out"] for c in range(NCORES)], axis=0)
    return out


def kernel(query_points, key_points, Wq, bq, Wk, bk):
    return run(query_points, key_points, Wq, bq, Wk, bk, trace=False)
